# revision 15
# baseline (speedup 1.0000x reference)
"""Single-query attention pooling + linear head, sharded batch-parallel
across 8 Trainium2 NeuronCores.

Reference computation (per batch b):
    score[s]  = sum_h inp[b,s,h] * q[b,h]
    score    -= 1e30 * (1 - mask)                (additive mask)
    att       = softmax(score)
    ext[b,h]  = sum_s att[s] * inp[b,s,h]
    ctrl[b,:] = W @ concat(q[b], ext[b]) + bias

Sharding: batch dim B=64 split 8 ways (8 batches/core); W and bias
replicated. No cross-device communication.

Per-core dataflow (positions s = p*32 + c, p = SBUF partition):
  - inp[b] staged in SBUF as one [128, 32, 256] tile per batch (4 MB),
    double-buffered so DMA of batch b+1 overlaps compute of batch b.
  - scores = per-chunk elementwise product with broadcast q plus a
    free-dim sum.  The ~17M lane-elements of product+reduction are
    load-balanced across DVE (products + batched binary-fold trees),
    GpSimd (products + fold trees) and ACT (Copy with fused
    accumulator).  Fold trees process a whole chunk-group per
    instruction via 3D APs, so instruction overhead stays negligible.
  - softmax with additive mask, pairwise-max tree + GpSimd partition
    all-reduce for the global max, ACT Exp with fused accumulation,
    1/denominator via exp(-ln d) + one Newton step.  (Several DVE op
    variants - tensor_tensor_reduce, tensor_reduce, 2-op tensor_scalar,
    reciprocal - misbehave on this HW path and are avoided.)
  - numerator via 32 accumulating float32r PE matmuls per batch
    (softmax-weight column stationary, inp chunk moving; float32r
    streams at full rate where fp32 needs two half-speed passes).
  - linear head via DVE product + ACT accumulate against replicated W.
"""

import numpy as np
from contextlib import ExitStack

import concourse.bacc as bacc
import concourse.mybir as mybir
import concourse.tile as tile
from concourse import bass_isa, bass_utils

P = 128          # SBUF partitions
C = 32           # seq chunks; position s = p*C + c
S = P * C        # 4096
H = 256
H2 = 2 * H
N_CORES = 8
B_TOTAL = 64
B = B_TOTAL // N_CORES   # batches per core

# score-work split (chunks per batch): products all on DVE; reductions
# split between ACT (Copy+accumulate pairs) and DVE (batched fold trees).
# GpSimd shares SBUF ports with DVE (exclusive lock), so it gets no
# streaming work - only its partition broadcast/all-reduce ops.
R_ACT, R_DVE = 16, 16             # reduction chunk-groups (sum = C)

F32 = mybir.dt.float32
F32R = mybir.dt.float32r
AF = mybir.ActivationFunctionType
ALU = mybir.AluOpType

_CACHE = {}


def _fold_tree(nc, eng, prod, sc, c0, k):
    """Binary-fold sum over the last dim of prod[:, 0:k, 0:256] using 3D
    APs (one instruction per level); final level writes sc[:, c0:c0+k]."""
    w = H // 2
    while w > 1:
        eng.tensor_tensor(
            out=prod[:, 0:k, 0:w], in0=prod[:, 0:k, 0:w], in1=prod[:, 0:k, w : 2 * w],
            op=ALU.add,
        )
        w //= 2
    eng.tensor_tensor(
        out=sc[:, c0 : c0 + k], in0=prod[:, 0:k, 0], in1=prod[:, 0:k, 1],
        op=ALU.add,
    )


def build_nc():
    nc = bacc.Bacc("TRN2", target_bir_lowering=False)

    inp = nc.dram_tensor("inp", [B, S, H], F32, kind="ExternalInput")
    msk = nc.dram_tensor("msk", [B, S], F32, kind="ExternalInput")
    qry = nc.dram_tensor("qry", [B, H], F32, kind="ExternalInput")
    wmat = nc.dram_tensor("wmat", [H, H2], F32, kind="ExternalInput")
    bvec = nc.dram_tensor("bvec", [H], F32, kind="ExternalInput")
    ext = nc.dram_tensor("ext", [B, H], F32, kind="ExternalOutput")
    ctl = nc.dram_tensor("ctl", [B, H], F32, kind="ExternalOutput")

    with ExitStack() as ctx:
        tc = ctx.enter_context(tile.TileContext(nc))
        const = ctx.enter_context(tc.tile_pool(name="const", bufs=1))
        inpp = ctx.enter_context(tc.tile_pool(name="inpp", bufs=2))
        prdp = ctx.enter_context(tc.tile_pool(name="prdp", bufs=2))
        smal = ctx.enter_context(tc.tile_pool(name="smal", bufs=2))
        scr = ctx.enter_context(tc.tile_pool(name="scr", bufs=3))
        psum = ctx.enter_context(tc.tile_pool(name="psum", bufs=2, space="PSUM"))

        # --- one-time loads (replicated weights + all-batch small inputs) ---
        w_sb = const.tile([P, 2, H2], F32)           # W[(g p), k] -> [p, g, k]
        nc.sync.dma_start(w_sb[:], wmat.rearrange("(g p) k -> p g k", p=P))
        b_sb = const.tile([P, 2], F32)               # bias[(g p)] -> [p, g]
        nc.sync.dma_start(b_sb[:], bvec.rearrange("(g p) -> p g", p=P))
        mk = const.tile([P, B, C], F32)              # mask[b, p*C+c] -> [p, b, c]
        nc.sync.dma_start(mk[:], msk.rearrange("b (p c) -> p b c", p=P))
        qrows = const.tile([1, B, H], F32)
        for b in range(B):
            nc.sync.dma_start(qrows[0:1, b, :], qry[b : b + 1, :])
        qrows4 = const.tile([1, B, 4, H], F32)       # q replicated 4x
        for b in range(B):
            for r in range(4):
                nc.sync.dma_start(qrows4[0:1, b, r, :], qry[b : b + 1, :])
        # additive mask: mask*1e30 - 1e30 -> {0 valid, -1e30 masked}
        m1 = const.tile([P, B, C], F32)
        nc.vector.tensor_scalar_mul(m1[:], mk[:], 1e30)
        nc.vector.tensor_scalar_add(m1[:], m1[:], -1e30)
        two = const.tile([P, 1], F32)                # constant 2.0 for NR step
        nc.scalar.activation(two[:], b_sb[:, 0:1], AF.Copy, bias=2.0, scale=0.0)

        for b in range(B):
            # --- stage inp[b]: [4096, 256] -> [128, 32, 256], s = p*32+c ---
            it = inpp.tile([P, C, H], F32, tag="it")
            src = inp[b].rearrange("(p c) h -> p c h", p=P)
            hc = C // 2
            nc.sync.dma_start(it[:, 0:hc, :], src[:, 0:hc, :])
            nc.sync.dma_start(it[:, hc:C, :], src[:, hc:C, :])

            qb4 = smal.tile([P, 4, H], F32, tag="qb4")
            nc.gpsimd.partition_broadcast(
                qb4[:].rearrange("p r h -> p (r h)"), qrows4[0:1, b, :, :]
            )

            # --- scores: products + reductions, balanced across engines ---
            # chunk c' reduce group: ACT [0,R_ACT), DVE [R_ACT,R_ACT+R_DVE),
            # GpSimd [R_ACT+R_DVE, C)
            sc = smal.tile([P, C], F32, tag="sc")
            pd = prdp.tile([P, R_DVE, H], F32, tag="pd")     # DVE-folded group
            pa = prdp.tile([P, R_ACT, H], F32, tag="pa")     # ACT-accumulated group
            dmp = scr.tile([P, H], F32, tag="dmp")

            for c0 in range(0, C, 4):
                dst = (
                    pa[:, c0 : c0 + 4, :]
                    if c0 < R_ACT
                    else pd[:, c0 - R_ACT : c0 - R_ACT + 4, :]
                )
                nc.vector.tensor_tensor(
                    out=dst, in0=it[:, c0 : c0 + 4, :], in1=qb4[:], op=ALU.mult
                )
            # reductions
            for j in range(R_ACT):
                nc.scalar.activation(
                    dmp[:], pa[:, j, :], AF.Copy, accum_out=sc[:, j : j + 1]
                )
            _fold_tree(nc, nc.vector, pd, sc, R_ACT, R_DVE)

            # --- additive mask + masked global max ---
            nc.vector.tensor_tensor(out=sc[:], in0=sc[:], in1=m1[:, b, :], op=ALU.add)
            mx = smal.tile([P, C], F32, tag="mx")
            w_ = C // 2
            nc.vector.tensor_tensor(
                out=mx[:, 0:w_], in0=sc[:, 0:w_], in1=sc[:, w_:C], op=ALU.max
            )
            while w_ > 1:
                h_ = w_ // 2
                nc.vector.tensor_tensor(
                    out=mx[:, 0:h_], in0=mx[:, 0:h_], in1=mx[:, h_:w_], op=ALU.max
                )
                w_ = h_
            gmaxb = smal.tile([P, 1], F32, tag="gmaxb")
            nc.gpsimd.partition_all_reduce(
                gmaxb[:], mx[:, 0:1], channels=P, reduce_op=bass_isa.ReduceOp.max
            )
            nmaxb = smal.tile([P, 1], F32, tag="nmaxb")
            nc.vector.tensor_scalar_mul(nmaxb[:], gmaxb[:], -1.0)
            # clamp at (gmax - 88) so the exp input range stays LUT-safe;
            # exp(-88) underflows to ~0 matching the reference's zero weights.
            clampb = smal.tile([P, 1], F32, tag="clampb")
            nc.vector.tensor_scalar_add(clampb[:], gmaxb[:], -88.0)
            nc.vector.tensor_scalar_max(sc[:], sc[:], clampb[:])

            # --- softmax weights + denominator ---
            wgt = smal.tile([P, C], F32, tag="wgt")
            part = smal.tile([P, 1], F32, tag="part")
            nc.scalar.activation(
                wgt[:], sc[:], AF.Exp, bias=nmaxb[:], scale=1.0, accum_out=part[:]
            )
            denb = smal.tile([P, 1], F32, tag="denb")
            nc.gpsimd.partition_all_reduce(
                denb[:], part[:], channels=P, reduce_op=bass_isa.ReduceOp.add
            )
            # 1/den = exp(-ln(den)), den in [1, 4096]; + one NR step:
            # r1 = r0 * (2 - den*r0)
            lden = smal.tile([P, 1], F32, tag="lden")
            nc.scalar.activation(lden[:], denb[:], AF.Ln)
            rd0 = smal.tile([P, 1], F32, tag="rd0")
            nc.scalar.activation(rd0[:], lden[:], AF.Exp, scale=-1.0)
            nrt = smal.tile([P, 1], F32, tag="nrt")
            nc.vector.tensor_tensor(out=nrt[:], in0=denb[:], in1=rd0[:], op=ALU.mult)
            nc.vector.tensor_tensor(out=nrt[:], in0=two[:], in1=nrt[:], op=ALU.subtract)
            rdenb = smal.tile([P, 1], F32, tag="rdenb")
            nc.vector.tensor_tensor(out=rdenb[:], in0=rd0[:], in1=nrt[:], op=ALU.mult)

            # --- numerator: [1,H] += wgt[:,c].T @ it[:,c,:] over chunks ---
            pnum = psum.tile([1, H], F32, tag="pnum")
            for c in range(C):
                nc.tensor.matmul(
                    pnum[:],
                    wgt[:, c : c + 1],
                    it[:, c, :],
                    start=(c == 0), stop=(c == C - 1),
                )

            extb = smal.tile([1, H], F32, tag="extb")
            nc.vector.tensor_scalar_mul(extb[:], pnum[:], rdenb[0:1, :])
            nc.sync.dma_start(ext[b : b + 1, :], extb[:])

            # --- linear head: ctrl = W @ [q; ext] + bias ---
            conc = smal.tile([P, H2], F32, tag="conc")
            nc.gpsimd.partition_broadcast(conc[:, 0:H], qrows[0:1, b, :])
            nc.gpsimd.partition_broadcast(conc[:, H:H2], extb[:])
            cdump = scr.tile([P, H2], F32, tag="cdump")
            cprod = scr.tile([P, H2], F32, tag="cprod")
            cc = smal.tile([P, 2], F32, tag="cc")
            for g in range(2):
                nc.vector.tensor_tensor(
                    out=cprod[:], in0=w_sb[:, g, :], in1=conc[:], op=ALU.mult
                )
                nc.scalar.activation(
                    cdump[:], cprod[:], AF.Copy, accum_out=cc[:, g : g + 1]
                )
            nc.vector.tensor_tensor(out=cc[:], in0=cc[:], in1=b_sb[:], op=ALU.add)
            nc.sync.dma_start(ctl[b].rearrange("(g p) -> p g", p=P), cc[:])

    nc.compile()
    return nc


def get_nc():
    if "nc" not in _CACHE:
        _CACHE["nc"] = build_nc()
    return _CACHE["nc"]


def make_in_maps(inp_seq, mask, query, W, b):
    inp_seq = np.ascontiguousarray(np.asarray(inp_seq, dtype=np.float32))
    mask = np.ascontiguousarray(np.asarray(mask, dtype=np.float32))
    query = np.ascontiguousarray(np.asarray(query, dtype=np.float32))
    W = np.ascontiguousarray(np.asarray(W, dtype=np.float32))
    b = np.ascontiguousarray(np.asarray(b, dtype=np.float32))
    in_maps = []
    for i in range(N_CORES):
        lo, hi = i * B, (i + 1) * B
        in_maps.append(
            {
                "inp": inp_seq[lo:hi],
                "msk": mask[lo:hi],
                "qry": query[lo:hi],
                "wmat": W,
                "bvec": b,
            }
        )
    return in_maps


def assemble(results):
    ext = np.concatenate([r["ext"] for r in results], axis=0)
    ctl = np.concatenate([r["ctl"] for r in results], axis=0)
    return ext.astype(np.float32), ctl.astype(np.float32)


def kernel(inp_seq, mask, query, W, b):
    nc = get_nc()
    in_maps = make_in_maps(inp_seq, mask, query, W, b)
    res = bass_utils.run_bass_kernel_spmd(nc, in_maps, core_ids=list(range(N_CORES)))
    return assemble(res.results)


# revision 16
# speedup vs baseline: 1.0794x; 1.0794x over previous
"""Single-query attention pooling + linear head, sharded batch-parallel
across 8 Trainium2 NeuronCores.

Reference computation (per batch b):
    score[s]  = sum_h inp[b,s,h] * q[b,h]
    score    -= 1e30 * (1 - mask)                (additive mask)
    att       = softmax(score)
    ext[b,h]  = sum_s att[s] * inp[b,s,h]
    ctrl[b,:] = W @ concat(q[b], ext[b]) + bias

Sharding: batch dim B=64 split 8 ways (8 batches/core); W and bias
replicated. No cross-device communication.

Per-core dataflow (positions s = p*32 + c, p = SBUF partition):
  - inp[b] staged in SBUF as one [128, 32, 256] tile per batch (4 MB),
    double-buffered so DMA of batch b+1 overlaps compute of batch b.
  - scores = per-chunk elementwise product with broadcast q plus a
    free-dim sum.  The ~17M lane-elements of product+reduction are
    load-balanced across DVE (products + batched binary-fold trees),
    GpSimd (products + fold trees) and ACT (Copy with fused
    accumulator).  Fold trees process a whole chunk-group per
    instruction via 3D APs, so instruction overhead stays negligible.
  - softmax with additive mask, pairwise-max tree + GpSimd partition
    all-reduce for the global max, ACT Exp with fused accumulation,
    1/denominator via exp(-ln d) + one Newton step.  (Several DVE op
    variants - tensor_tensor_reduce, tensor_reduce, 2-op tensor_scalar,
    reciprocal - misbehave on this HW path and are avoided.)
  - numerator via 32 accumulating float32r PE matmuls per batch
    (softmax-weight column stationary, inp chunk moving; float32r
    streams at full rate where fp32 needs two half-speed passes).
  - linear head via DVE product + ACT accumulate against replicated W.
"""

import numpy as np
from contextlib import ExitStack

import concourse.bacc as bacc
import concourse.mybir as mybir
import concourse.tile as tile
from concourse import bass_isa, bass_utils

P = 128          # SBUF partitions
C = 32           # seq chunks; position s = p*C + c
S = P * C        # 4096
H = 256
H2 = 2 * H
N_CORES = 8
B_TOTAL = 64
B = B_TOTAL // N_CORES   # batches per core

# score-work split (chunks per batch): products all on DVE; reductions
# split between ACT (Copy+accumulate pairs) and DVE (batched fold trees).
# GpSimd shares SBUF ports with DVE (exclusive lock), so it gets no
# streaming work - only its partition broadcast/all-reduce ops.
R_ACT, R_DVE = 12, 20             # reduction chunk-groups (sum = C)

F32 = mybir.dt.float32
F32R = mybir.dt.float32r
AF = mybir.ActivationFunctionType
ALU = mybir.AluOpType

_CACHE = {}


def _fold_tree(nc, eng, prod, sc, c0, k):
    """Binary-fold sum over the last dim of prod[:, 0:k, 0:256] using 3D
    APs (one instruction per level); final level writes sc[:, c0:c0+k]."""
    w = H // 2
    while w > 1:
        eng.tensor_tensor(
            out=prod[:, 0:k, 0:w], in0=prod[:, 0:k, 0:w], in1=prod[:, 0:k, w : 2 * w],
            op=ALU.add,
        )
        w //= 2
    eng.tensor_tensor(
        out=sc[:, c0 : c0 + k], in0=prod[:, 0:k, 0], in1=prod[:, 0:k, 1],
        op=ALU.add,
    )


def build_nc():
    nc = bacc.Bacc("TRN2", target_bir_lowering=False)

    inp = nc.dram_tensor("inp", [B, S, H], F32, kind="ExternalInput")
    msk = nc.dram_tensor("msk", [B, S], F32, kind="ExternalInput")
    qry = nc.dram_tensor("qry", [B, H], F32, kind="ExternalInput")
    wmat = nc.dram_tensor("wmat", [H, H2], F32, kind="ExternalInput")
    bvec = nc.dram_tensor("bvec", [H], F32, kind="ExternalInput")
    ext = nc.dram_tensor("ext", [B, H], F32, kind="ExternalOutput")
    ctl = nc.dram_tensor("ctl", [B, H], F32, kind="ExternalOutput")

    with ExitStack() as ctx:
        tc = ctx.enter_context(tile.TileContext(nc))
        const = ctx.enter_context(tc.tile_pool(name="const", bufs=1))
        inpp = ctx.enter_context(tc.tile_pool(name="inpp", bufs=2))
        prdp = ctx.enter_context(tc.tile_pool(name="prdp", bufs=2))
        smal = ctx.enter_context(tc.tile_pool(name="smal", bufs=2))
        scr = ctx.enter_context(tc.tile_pool(name="scr", bufs=3))
        psum = ctx.enter_context(tc.tile_pool(name="psum", bufs=2, space="PSUM"))

        # --- one-time loads (replicated weights + all-batch small inputs) ---
        w_sb = const.tile([P, 2, H2], F32)           # W[(g p), k] -> [p, g, k]
        nc.sync.dma_start(w_sb[:], wmat.rearrange("(g p) k -> p g k", p=P))
        b_sb = const.tile([P, 2], F32)               # bias[(g p)] -> [p, g]
        nc.sync.dma_start(b_sb[:], bvec.rearrange("(g p) -> p g", p=P))
        mk = const.tile([P, B, C], F32)              # mask[b, p*C+c] -> [p, b, c]
        nc.sync.dma_start(mk[:], msk.rearrange("b (p c) -> p b c", p=P))
        qrows = const.tile([1, B, H], F32)
        for b in range(B):
            nc.sync.dma_start(qrows[0:1, b, :], qry[b : b + 1, :])
        qrows4 = const.tile([1, B, 4, H], F32)       # q replicated 4x
        for b in range(B):
            for r in range(4):
                nc.sync.dma_start(qrows4[0:1, b, r, :], qry[b : b + 1, :])
        # additive mask: mask*1e30 - 1e30 -> {0 valid, -1e30 masked}
        m1 = const.tile([P, B, C], F32)
        nc.vector.tensor_scalar_mul(m1[:], mk[:], 1e30)
        nc.vector.tensor_scalar_add(m1[:], m1[:], -1e30)
        two = const.tile([P, 1], F32)                # constant 2.0 for NR step
        nc.scalar.activation(two[:], b_sb[:, 0:1], AF.Copy, bias=2.0, scale=0.0)

        for b in range(B):
            # --- stage inp[b]: [4096, 256] -> [128, 32, 256], s = p*32+c ---
            it = inpp.tile([P, C, H], F32R, tag="it")
            src = inp[b].rearrange("(p c) h -> p c h", p=P).bitcast(F32R)
            hc = C // 2
            nc.sync.dma_start(it[:, 0:hc, :], src[:, 0:hc, :])
            nc.sync.dma_start(it[:, hc:C, :], src[:, hc:C, :])

            qb4 = smal.tile([P, 4, H], F32, tag="qb4")
            nc.gpsimd.partition_broadcast(
                qb4[:].rearrange("p r h -> p (r h)"), qrows4[0:1, b, :, :]
            )

            # --- scores: products + reductions, balanced across engines ---
            # chunk c' reduce group: ACT [0,R_ACT), DVE [R_ACT,R_ACT+R_DVE),
            # GpSimd [R_ACT+R_DVE, C)
            sc = smal.tile([P, C], F32, tag="sc")
            pd = prdp.tile([P, R_DVE, H], F32, tag="pd")     # DVE-folded group
            pa = prdp.tile([P, R_ACT, H], F32, tag="pa")     # ACT-accumulated group
            dmp = scr.tile([P, H], F32, tag="dmp")

            for c0 in range(0, C, 4):
                dst = (
                    pa[:, c0 : c0 + 4, :]
                    if c0 < R_ACT
                    else pd[:, c0 - R_ACT : c0 - R_ACT + 4, :]
                )
                nc.vector.tensor_tensor(
                    out=dst, in0=it[:, c0 : c0 + 4, :].bitcast(F32), in1=qb4[:],
                    op=ALU.mult,
                )
            # reductions
            for j in range(R_ACT):
                nc.scalar.activation(
                    dmp[:], pa[:, j, :], AF.Copy, accum_out=sc[:, j : j + 1]
                )
            _fold_tree(nc, nc.vector, pd, sc, R_ACT, R_DVE)

            # --- additive mask + masked global max ---
            nc.vector.tensor_tensor(out=sc[:], in0=sc[:], in1=m1[:, b, :], op=ALU.add)
            mx = smal.tile([P, C], F32, tag="mx")
            w_ = C // 2
            nc.vector.tensor_tensor(
                out=mx[:, 0:w_], in0=sc[:, 0:w_], in1=sc[:, w_:C], op=ALU.max
            )
            while w_ > 1:
                h_ = w_ // 2
                nc.vector.tensor_tensor(
                    out=mx[:, 0:h_], in0=mx[:, 0:h_], in1=mx[:, h_:w_], op=ALU.max
                )
                w_ = h_
            gmaxb = smal.tile([P, 1], F32, tag="gmaxb")
            nc.gpsimd.partition_all_reduce(
                gmaxb[:], mx[:, 0:1], channels=P, reduce_op=bass_isa.ReduceOp.max
            )
            nmaxb = smal.tile([P, 1], F32, tag="nmaxb")
            nc.vector.tensor_scalar_mul(nmaxb[:], gmaxb[:], -1.0)
            # clamp at (gmax - 88) so the exp input range stays LUT-safe;
            # exp(-88) underflows to ~0 matching the reference's zero weights.
            clampb = smal.tile([P, 1], F32, tag="clampb")
            nc.vector.tensor_scalar_add(clampb[:], gmaxb[:], -88.0)
            nc.vector.tensor_scalar_max(sc[:], sc[:], clampb[:])

            # --- softmax weights + denominator ---
            wgt = smal.tile([P, C], F32, tag="wgt")
            part = smal.tile([P, 1], F32, tag="part")
            nc.scalar.activation(
                wgt[:], sc[:], AF.Exp, bias=nmaxb[:], scale=1.0, accum_out=part[:]
            )
            denb = smal.tile([P, 1], F32, tag="denb")
            nc.gpsimd.partition_all_reduce(
                denb[:], part[:], channels=P, reduce_op=bass_isa.ReduceOp.add
            )
            # 1/den = exp(-ln(den)), den in [1, 4096]; + one NR step:
            # r1 = r0 * (2 - den*r0)
            lden = smal.tile([P, 1], F32, tag="lden")
            nc.scalar.activation(lden[:], denb[:], AF.Ln)
            rd0 = smal.tile([P, 1], F32, tag="rd0")
            nc.scalar.activation(rd0[:], lden[:], AF.Exp, scale=-1.0)
            nrt = smal.tile([P, 1], F32, tag="nrt")
            nc.vector.tensor_tensor(out=nrt[:], in0=denb[:], in1=rd0[:], op=ALU.mult)
            nc.vector.tensor_tensor(out=nrt[:], in0=two[:], in1=nrt[:], op=ALU.subtract)
            rdenb = smal.tile([P, 1], F32, tag="rdenb")
            nc.vector.tensor_tensor(out=rdenb[:], in0=rd0[:], in1=nrt[:], op=ALU.mult)

            # --- numerator: [1,H] += wgt[:,c].T @ it[:,c,:] over chunks ---
            wgtr = smal.tile([P, C], F32R, tag="wgtr")
            nc.vector.tensor_copy(wgtr[:], wgt[:])
            pnum = psum.tile([1, H], F32, tag="pnum")
            for c in range(C):
                nc.tensor.matmul(
                    pnum[:],
                    wgtr[:, c : c + 1],
                    it[:, c, :],
                    start=(c == 0), stop=(c == C - 1),
                )

            extb = smal.tile([1, H], F32, tag="extb")
            nc.vector.tensor_scalar_mul(extb[:], pnum[:], rdenb[0:1, :])
            nc.sync.dma_start(ext[b : b + 1, :], extb[:])

            # --- linear head: ctrl = W @ [q; ext] + bias ---
            conc = smal.tile([P, H2], F32, tag="conc")
            nc.gpsimd.partition_broadcast(conc[:, 0:H], qrows[0:1, b, :])
            nc.gpsimd.partition_broadcast(conc[:, H:H2], extb[:])
            cdump = scr.tile([P, H2], F32, tag="cdump")
            cprod = scr.tile([P, H2], F32, tag="cprod")
            cc = smal.tile([P, 2], F32, tag="cc")
            for g in range(2):
                nc.vector.tensor_tensor(
                    out=cprod[:], in0=w_sb[:, g, :], in1=conc[:], op=ALU.mult
                )
                nc.scalar.activation(
                    cdump[:], cprod[:], AF.Copy, accum_out=cc[:, g : g + 1]
                )
            nc.vector.tensor_tensor(out=cc[:], in0=cc[:], in1=b_sb[:], op=ALU.add)
            nc.sync.dma_start(ctl[b].rearrange("(g p) -> p g", p=P), cc[:])

    nc.compile()
    return nc


def get_nc():
    if "nc" not in _CACHE:
        _CACHE["nc"] = build_nc()
    return _CACHE["nc"]


def make_in_maps(inp_seq, mask, query, W, b):
    inp_seq = np.ascontiguousarray(np.asarray(inp_seq, dtype=np.float32))
    mask = np.ascontiguousarray(np.asarray(mask, dtype=np.float32))
    query = np.ascontiguousarray(np.asarray(query, dtype=np.float32))
    W = np.ascontiguousarray(np.asarray(W, dtype=np.float32))
    b = np.ascontiguousarray(np.asarray(b, dtype=np.float32))
    in_maps = []
    for i in range(N_CORES):
        lo, hi = i * B, (i + 1) * B
        in_maps.append(
            {
                "inp": inp_seq[lo:hi],
                "msk": mask[lo:hi],
                "qry": query[lo:hi],
                "wmat": W,
                "bvec": b,
            }
        )
    return in_maps


def assemble(results):
    ext = np.concatenate([r["ext"] for r in results], axis=0)
    ctl = np.concatenate([r["ctl"] for r in results], axis=0)
    return ext.astype(np.float32), ctl.astype(np.float32)


def kernel(inp_seq, mask, query, W, b):
    nc = get_nc()
    in_maps = make_in_maps(inp_seq, mask, query, W, b)
    res = bass_utils.run_bass_kernel_spmd(nc, in_maps, core_ids=list(range(N_CORES)))
    return assemble(res.results)


# revision 20
# speedup vs baseline: 1.5647x; 1.4496x over previous
"""Single-query attention pooling + linear head, sharded batch-parallel
across 8 Trainium2 NeuronCores.

Reference computation (per batch b):
    score[s]  = sum_h inp[b,s,h] * q[b,h]
    score    -= 1e30 * (1 - mask)                (additive mask)
    att       = softmax(score)
    ext[b,h]  = sum_s att[s] * inp[b,s,h]
    ctrl[b,:] = W @ concat(q[b], ext[b]) + bias

Sharding: batch dim B=64 split 8 ways (8 batches/core); W and bias
replicated. No cross-device communication.

Per-core dataflow (positions s = p*32 + c, p = SBUF partition):
  - inp[b] staged in SBUF as one [128, 32, 256] tile per batch (4 MB),
    double-buffered so DMA of batch b+1 overlaps compute of batch b.
  - scores = per-chunk elementwise product with broadcast q plus a
    free-dim sum.  The ~17M lane-elements of product+reduction are
    load-balanced across DVE (products + batched binary-fold trees),
    GpSimd (products + fold trees) and ACT (Copy with fused
    accumulator).  Fold trees process a whole chunk-group per
    instruction via 3D APs, so instruction overhead stays negligible.
  - softmax with additive mask, pairwise-max tree + GpSimd partition
    all-reduce for the global max, ACT Exp with fused accumulation,
    1/denominator via exp(-ln d) + one Newton step.  (Several DVE op
    variants - tensor_tensor_reduce, tensor_reduce, 2-op tensor_scalar,
    reciprocal - misbehave on this HW path and are avoided.)
  - numerator via 32 accumulating float32r PE matmuls per batch
    (softmax-weight column stationary, inp chunk moving; float32r
    streams at full rate where fp32 needs two half-speed passes).
  - linear head via DVE product + ACT accumulate against replicated W.
"""

import numpy as np
from contextlib import ExitStack

import concourse.bacc as bacc
import concourse.mybir as mybir
import concourse.tile as tile
from concourse import bass_isa, bass_utils

P = 128          # SBUF partitions
C = 32           # seq chunks; position s = p*C + c
S = P * C        # 4096
H = 256
H2 = 2 * H
N_CORES = 8
B_TOTAL = 64
B = B_TOTAL // N_CORES   # batches per core

# score-work split (chunks per batch): products all on DVE; reductions
# split between ACT (Copy+accumulate pairs) and DVE (batched fold trees).
# GpSimd shares SBUF ports with DVE (exclusive lock), so it gets no
# streaming work - only its partition broadcast/all-reduce ops.
R_ACT, R_DVE = 12, 20             # reduction chunk-groups (sum = C)

F32 = mybir.dt.float32
F32R = mybir.dt.float32r
AF = mybir.ActivationFunctionType
ALU = mybir.AluOpType

_CACHE = {}


def _fold_tree(nc, eng, prod, sc, c0, k):
    """Binary-fold sum over the last dim of prod[:, 0:k, 0:256] using 3D
    APs (one instruction per level); final level writes sc[:, c0:c0+k]."""
    w = H // 2
    while w > 1:
        eng.tensor_tensor(
            out=prod[:, 0:k, 0:w], in0=prod[:, 0:k, 0:w], in1=prod[:, 0:k, w : 2 * w],
            op=ALU.add,
        )
        w //= 2
    eng.tensor_tensor(
        out=sc[:, c0 : c0 + k], in0=prod[:, 0:k, 0], in1=prod[:, 0:k, 1],
        op=ALU.add,
    )


def build_nc():
    nc = bacc.Bacc("TRN2", target_bir_lowering=False)

    inp = nc.dram_tensor("inp", [B, S, H], F32, kind="ExternalInput")
    msk = nc.dram_tensor("msk", [B, S], F32, kind="ExternalInput")
    qry = nc.dram_tensor("qry", [B, H], F32, kind="ExternalInput")
    wmat = nc.dram_tensor("wmat", [H, H2], F32, kind="ExternalInput")
    bvec = nc.dram_tensor("bvec", [H], F32, kind="ExternalInput")
    ext = nc.dram_tensor("ext", [B, H], F32, kind="ExternalOutput")
    ctl = nc.dram_tensor("ctl", [B, H], F32, kind="ExternalOutput")

    with ExitStack() as ctx:
        tc = ctx.enter_context(tile.TileContext(nc))
        const = ctx.enter_context(tc.tile_pool(name="const", bufs=1))
        inpp = ctx.enter_context(tc.tile_pool(name="inpp", bufs=3))
        prdp = ctx.enter_context(tc.tile_pool(name="prdp", bufs=2))
        smal = ctx.enter_context(tc.tile_pool(name="smal", bufs=4))
        qbp = ctx.enter_context(tc.tile_pool(name="qbp", bufs=2))
        scr = ctx.enter_context(tc.tile_pool(name="scr", bufs=2))
        psum = ctx.enter_context(tc.tile_pool(name="psum", bufs=4, space="PSUM"))

        # --- one-time loads (replicated weights + all-batch small inputs) ---
        w_sb = const.tile([P, 2, H2], F32)           # W[(g p), k] -> [p, g, k]
        nc.sync.dma_start(w_sb[:], wmat.rearrange("(g p) k -> p g k", p=P))
        b_sb = const.tile([P, 2], F32)               # bias[(g p)] -> [p, g]
        nc.sync.dma_start(b_sb[:], bvec.rearrange("(g p) -> p g", p=P))
        mk = const.tile([P, B, C], F32)              # mask[b, p*C+c] -> [p, b, c]
        nc.sync.dma_start(mk[:], msk.rearrange("b (p c) -> p b c", p=P))
        qrows = const.tile([1, B, H], F32)
        for b in range(B):
            nc.sync.dma_start(qrows[0:1, b, :], qry[b : b + 1, :])
        # additive mask: mask*1e30 - 1e30 -> {0 valid, -1e30 masked}
        m1 = const.tile([P, B, C], F32)
        nc.vector.tensor_scalar_mul(m1[:], mk[:], 1e30)
        nc.vector.tensor_scalar_add(m1[:], m1[:], -1e30)
        two = const.tile([P, 1], F32)                # constant 2.0 for NR step
        nc.scalar.activation(two[:], b_sb[:, 0:1], AF.Copy, bias=2.0, scale=0.0)

        for b in range(B):
            # --- stage inp[b]: [4096, 256] -> [128, 32, 256], s = p*32+c ---
            hc = C // 2
            it0 = inpp.tile([P, hc, H], F32R, tag="it0")
            it1 = inpp.tile([P, hc, H], F32R, tag="it1")
            src = inp[b].rearrange("(p c) h -> p c h", p=P).bitcast(F32R)
            nc.sync.dma_start(it0[:], src[:, 0:hc, :])
            nc.sync.dma_start(it1[:], src[:, hc:C, :])

            def itc(c):
                return it0[:, c, :] if c < hc else it1[:, c - hc, :]

            def itc4(c0):
                t = it0 if c0 < hc else it1
                cb = c0 if c0 < hc else c0 - hc
                return t[:, cb : cb + 4, :]

            qb4 = qbp.tile([P, 4, H], F32, tag="qb4")
            for r in range(4):
                nc.gpsimd.partition_broadcast(qb4[:, r, :], qrows[0:1, b, :])

            # --- scores: products + reductions, balanced across engines ---
            # chunk c' reduce group: ACT [0,R_ACT), DVE [R_ACT,R_ACT+R_DVE),
            # GpSimd [R_ACT+R_DVE, C)
            sc = smal.tile([P, C], F32, tag="sc")
            pd = prdp.tile([P, R_DVE, H], F32, tag="pd")     # DVE-folded group
            pa = prdp.tile([P, R_ACT, H], F32, tag="pa")     # ACT-accumulated group
            dmp = scr.tile([P, H], F32, tag="dmp")

            for c0 in range(0, C, 4):
                dst = (
                    pa[:, c0 : c0 + 4, :]
                    if c0 < R_ACT
                    else pd[:, c0 - R_ACT : c0 - R_ACT + 4, :]
                )
                nc.vector.tensor_tensor(
                    out=dst, in0=itc4(c0).bitcast(F32), in1=qb4[:],
                    op=ALU.mult,
                )
            # reductions
            for j in range(R_ACT):
                nc.scalar.activation(
                    dmp[:], pa[:, j, :], AF.Copy, accum_out=sc[:, j : j + 1]
                )
            _fold_tree(nc, nc.vector, pd, sc, R_ACT, R_DVE)

            # --- additive mask + masked global max ---
            nc.vector.tensor_tensor(out=sc[:], in0=sc[:], in1=m1[:, b, :], op=ALU.add)
            mx = smal.tile([P, C], F32, tag="mx")
            w_ = C // 2
            nc.vector.tensor_tensor(
                out=mx[:, 0:w_], in0=sc[:, 0:w_], in1=sc[:, w_:C], op=ALU.max
            )
            while w_ > 1:
                h_ = w_ // 2
                nc.vector.tensor_tensor(
                    out=mx[:, 0:h_], in0=mx[:, 0:h_], in1=mx[:, h_:w_], op=ALU.max
                )
                w_ = h_
            gmaxb = smal.tile([P, 1], F32, tag="gmaxb")
            nc.gpsimd.partition_all_reduce(
                gmaxb[:], mx[:, 0:1], channels=P, reduce_op=bass_isa.ReduceOp.max
            )
            nmaxb = smal.tile([P, 1], F32, tag="nmaxb")
            nc.vector.tensor_scalar_mul(nmaxb[:], gmaxb[:], -1.0)
            # clamp at (gmax - 88) so the exp input range stays LUT-safe;
            # exp(-88) underflows to ~0 matching the reference's zero weights.
            clampb = smal.tile([P, 1], F32, tag="clampb")
            nc.vector.tensor_scalar_add(clampb[:], gmaxb[:], -88.0)
            nc.vector.tensor_scalar_max(sc[:], sc[:], clampb[:])

            # --- softmax weights + denominator ---
            wgt = smal.tile([P, C], F32, tag="wgt")
            part = smal.tile([P, 1], F32, tag="part")
            nc.scalar.activation(
                wgt[:], sc[:], AF.Exp, bias=nmaxb[:], scale=1.0, accum_out=part[:]
            )
            denb = smal.tile([P, 1], F32, tag="denb")
            nc.gpsimd.partition_all_reduce(
                denb[:], part[:], channels=P, reduce_op=bass_isa.ReduceOp.add
            )
            # 1/den = exp(-ln(den)), den in [1, 4096]; + one NR step:
            # r1 = r0 * (2 - den*r0)
            lden = smal.tile([P, 1], F32, tag="lden")
            nc.scalar.activation(lden[:], denb[:], AF.Ln)
            rd0 = smal.tile([P, 1], F32, tag="rd0")
            nc.scalar.activation(rd0[:], lden[:], AF.Exp, scale=-1.0)
            nrt = smal.tile([P, 1], F32, tag="nrt")
            nc.vector.tensor_tensor(out=nrt[:], in0=denb[:], in1=rd0[:], op=ALU.mult)
            nc.vector.tensor_tensor(out=nrt[:], in0=two[:], in1=nrt[:], op=ALU.subtract)
            rdenb = smal.tile([P, 1], F32, tag="rdenb")
            nc.vector.tensor_tensor(out=rdenb[:], in0=rd0[:], in1=nrt[:], op=ALU.mult)

            # --- numerator: [1,H] += wgt[:,c].T @ it[:,c,:] over chunks ---
            wgtr = smal.tile([P, C], F32R, tag="wgtr")
            nc.vector.tensor_copy(wgtr[:], wgt[:])
            pnum = psum.tile([1, H], F32, tag="pnum")
            for c in range(C):
                nc.tensor.matmul(
                    pnum[:],
                    wgtr[:, c : c + 1],
                    itc(c),
                    start=(c == 0), stop=(c == C - 1),
                )

            extb = smal.tile([1, H], F32, tag="extb")
            nc.vector.tensor_scalar_mul(extb[:], pnum[:], rdenb[0:1, :])
            nc.sync.dma_start(ext[b : b + 1, :], extb[:])

            # --- linear head: ctrl = W @ [q; ext] + bias ---
            conc = smal.tile([P, H2], F32, tag="conc")
            nc.gpsimd.partition_broadcast(conc[:, 0:H], qrows[0:1, b, :])
            nc.gpsimd.partition_broadcast(conc[:, H:H2], extb[:])
            cdump = scr.tile([P, H2], F32, tag="cdump")
            cprod = scr.tile([P, H2], F32, tag="cprod")
            cc = smal.tile([P, 2], F32, tag="cc")
            for g in range(2):
                nc.vector.tensor_tensor(
                    out=cprod[:], in0=w_sb[:, g, :], in1=conc[:], op=ALU.mult
                )
                nc.scalar.activation(
                    cdump[:], cprod[:], AF.Copy, accum_out=cc[:, g : g + 1]
                )
            nc.vector.tensor_tensor(out=cc[:], in0=cc[:], in1=b_sb[:], op=ALU.add)
            nc.sync.dma_start(ctl[b].rearrange("(g p) -> p g", p=P), cc[:])

    nc.compile()
    return nc


def get_nc():
    if "nc" not in _CACHE:
        _CACHE["nc"] = build_nc()
    return _CACHE["nc"]


def make_in_maps(inp_seq, mask, query, W, b):
    inp_seq = np.ascontiguousarray(np.asarray(inp_seq, dtype=np.float32))
    mask = np.ascontiguousarray(np.asarray(mask, dtype=np.float32))
    query = np.ascontiguousarray(np.asarray(query, dtype=np.float32))
    W = np.ascontiguousarray(np.asarray(W, dtype=np.float32))
    b = np.ascontiguousarray(np.asarray(b, dtype=np.float32))
    in_maps = []
    for i in range(N_CORES):
        lo, hi = i * B, (i + 1) * B
        in_maps.append(
            {
                "inp": inp_seq[lo:hi],
                "msk": mask[lo:hi],
                "qry": query[lo:hi],
                "wmat": W,
                "bvec": b,
            }
        )
    return in_maps


def assemble(results):
    ext = np.concatenate([r["ext"] for r in results], axis=0)
    ctl = np.concatenate([r["ctl"] for r in results], axis=0)
    return ext.astype(np.float32), ctl.astype(np.float32)


def kernel(inp_seq, mask, query, W, b):
    nc = get_nc()
    in_maps = make_in_maps(inp_seq, mask, query, W, b)
    res = bass_utils.run_bass_kernel_spmd(nc, in_maps, core_ids=list(range(N_CORES)))
    return assemble(res.results)
